# revision 1
# baseline (speedup 1.0000x reference)
"""C2Q (BiDAF-style) attention kernel for 8 TRN2 NeuronCores.

Pure data parallel: 64 batches sharded 8-per-core. Per batch b (reference):
    S = c @ c_w + (q @ q_w)^T + (c * cq_w) @ q^T + bias      (1024, 128)
    S1 = masked_softmax(S, q_mask, axis=j)
    S2 = masked_softmax(S1, c_mask, axis=i)
    A = S1 @ q ; Bm = S1 @ (S2^T @ c)
    out = [c | A | c*A | c*Bm]                                (1024, 512)

Key algebra: softmax over j is invariant to per-i constants, so the
c @ c_w term and the scalar bias CANCEL in S1 and never need computing.
Only R[j] = q @ q_w + log-mask(q_mask) survives (per-j), and it is a
per-partition bias in the transposed domain.

Device-side formulation (per batch):
    S^T[j,i] = qmodT.T @ cT   (bf16, 2 matmuls of N=512, qmodT stationary)
    E0T      = exp(S^T + R[j])              # ACT bias; stored bf16 [j, 1024]
    per chunk k: E0 = transpose(E0T chunk)  # PE; bf16 psum
                 E0_f32 -> SBUF with rowsum[i] via DVE accum_out
    rcprow = 1/rowsum ; G = exp(E0*rcprow + cmb[i])   # ACT scale+bias APs
    Traw[j,0:129] = sum_k G_k^T @ [c_k | 1]  (col 128 = colsum)
    Ts = Traw * (1/colsum)  -> bf16, next to q in the qq tile
    [Araw|Braw] = E0T_k.T @ [q | Ts]   (bf16, N=256)
    out chunk = [c | Araw*rcprow | c*Araw*rcprow | c*Braw*rcprow]
No max-subtraction needed: |S+R| <= ~30 so exp stays in range.
"""

import os
import numpy as np
import ml_dtypes

import concourse.bass as bass
import concourse.tile as tile
from concourse import bacc, mybir
from concourse.bass_utils import run_bass_kernel_spmd

F32 = mybir.dt.float32
BF16 = mybir.dt.bfloat16
AF = mybir.ActivationFunctionType
ALU = mybir.AluOpType

N_CORES = 8
B, CL, QL, D = 64, 1024, 128, 128
BPC = B // N_CORES          # batches per core
NK = CL // 128              # 128-row chunks per batch
MASK_NEG = -50.0            # exp(-50+eps) vanishes in f32 sums; in ACT range

LAST_RESULTS = None         # set by kernel() for test.py profiling


def _build_graph(loop_n=0):
    """loop_n=0: straight-line graph (production). loop_n=N>0: wrap the whole
    computation in a hardware For_i loop repeating it N times (timing only)."""
    nc = bacc.Bacc()

    c_ext = nc.declare_dram_parameter("c", [BPC, CL, D], F32, isOutput=False)
    cT_ext = nc.declare_dram_parameter("cT", [BPC, D, CL], BF16, isOutput=False)
    qq_ext = nc.declare_dram_parameter("qq", [BPC, D, 2 * QL], BF16, isOutput=False)
    cmR_ext = nc.declare_dram_parameter("cmR", [128, BPC * NK + BPC], F32, isOutput=False)
    id_ext = nc.declare_dram_parameter("ident", [128, 128], BF16, isOutput=False)
    out_ext = nc.declare_dram_parameter("out", [BPC, CL, 4 * D], F32, isOutput=True)

    with tile.TileContext(nc) as tc:
        with (
            tc.tile_pool(name="const", bufs=1) as const,
            tc.tile_pool(name="cbuf", bufs=3) as cbuf,
            tc.tile_pool(name="ctbuf", bufs=2) as ctbuf,
            tc.tile_pool(name="qq", bufs=2) as qqp,
            tc.tile_pool(name="e0tp", bufs=2) as e0tp,
            tc.tile_pool(name="e0p", bufs=12) as e0p,
            tc.tile_pool(name="gp", bufs=3) as gp,
            tc.tile_pool(name="stg", bufs=4) as stg,
            tc.tile_pool(name="rsp", bufs=2) as rsp,
            tc.tile_pool(name="stp", bufs=2, space=bass.MemorySpace.PSUM) as stp,
            tc.tile_pool(name="tpp", bufs=2, space=bass.MemorySpace.PSUM) as tpp,
            tc.tile_pool(name="trawp", bufs=1, space=bass.MemorySpace.PSUM) as trawp,
            tc.tile_pool(name="abp", bufs=3, space=bass.MemorySpace.PSUM) as abp,
        ):
            ident = const.tile([128, 128], BF16, tag="ident")
            nc.sync.dma_start(ident[:], id_ext[:])
            cmR = const.tile([128, BPC * NK + BPC], F32, tag="cmR")
            nc.sync.dma_start(cmR[:], cmR_ext[:])

            def _batch(b):
                # c tile: 8 groups of [128 ctx cols | ones col] -> [128, 8*129]
                c_t = cbuf.tile([128, NK * 129], F32, tag="c")
                cg = c_t[:].rearrange("p (k d) -> p k d", d=129)
                nc.vector.memset(cg[:, :, 128:129], 1.0)
                nc.sync.dma_start(
                    cg[:, :, 0:128],
                    c_ext[b].rearrange("(k p) d -> p k d", p=128),
                )
                cT_t = ctbuf.tile([128, CL], BF16, tag="cT")
                nc.sync.dma_start(cT_t[:], cT_ext[b])
                # qq tile: [qmodT | q | Ts]  (bf16)
                qq_t = qqp.tile([128, 3 * QL], BF16, tag="qq")
                nc.sync.dma_start(qq_t[:, 0:2 * QL], qq_ext[b])

                rowsum = rsp.tile([128, NK], F32, tag="rowsum")
                rcprow = rsp.tile([128, NK], F32, tag="rcprow")
                rcp2 = rsp.tile([128, 1], F32, tag="rcp2")

                # S^T = qmodT.T @ cT ; E0T = exp(S^T + R[j])  [j, 1024] bf16
                e0t_t = e0tp.tile([128, CL], BF16, tag="e0t")
                for h in range(2):
                    sp = stp.tile([128, 512], F32, tag="sp")
                    nc.tensor.matmul(
                        sp[:], qq_t[:, 0:QL], cT_t[:, h * 512:(h + 1) * 512]
                    )
                    nc.scalar.activation(
                        e0t_t[:, h * 512:(h + 1) * 512], sp[:], AF.Exp,
                        bias=cmR[:, BPC * NK + b:BPC * NK + b + 1],
                    )

                # per chunk: E0 natural (f32) + rowsum via DVE accum
                e0_l = []
                for k in range(NK):
                    ep = tpp.tile([128, 128], BF16, tag="ep")
                    nc.tensor.transpose(ep[:], e0t_t[:, k * 128:(k + 1) * 128], ident[:])
                    e0_t = e0p.tile([128, 128], F32, tag="e0")
                    nc.vector.tensor_scalar(
                        e0_t[:], ep[:], 1.0, 0.0, ALU.mult, ALU.add,
                        accum_out=rowsum[:, k:k + 1],
                    )
                    e0_l.append(e0_t)
                nc.vector.reciprocal(rcprow[:], rowsum[:])

                # G = exp(E0*rcprow + cmb) ; Traw accum (col 128 = colsum)
                traw = trawp.tile([128, 129], F32, tag="traw")
                for k in range(NK):
                    col = b * NK + k
                    g_t = gp.tile([128, QL], F32, tag="g")
                    nc.scalar.activation(
                        g_t[:], e0_l[k][:], AF.Exp,
                        bias=cmR[:, col:col + 1],
                        scale=rcprow[:, k:k + 1],
                    )
                    nc.tensor.matmul(
                        traw[:, 0:129], g_t[:], c_t[:, k * 129:(k + 1) * 129],
                        start=(k == 0), stop=(k == NK - 1),
                    )
                nc.vector.reciprocal(rcp2[:], traw[:, 128:129])
                nc.vector.tensor_scalar_mul(qq_t[:, 2 * QL:3 * QL], traw[:, 0:128], rcp2[:])

                # [Araw|Braw] = E0T_k.T @ [q | Ts] ; stage output chunk
                for k in range(NK):
                    ab = abp.tile([128, 2 * QL], F32, tag="ab")
                    nc.tensor.matmul(
                        ab[:], e0t_t[:, k * 128:(k + 1) * 128], qq_t[:, QL:3 * QL]
                    )
                    st = stg.tile([128, 4 * D], F32, tag="st")
                    cchunk = c_t[:, k * 129:k * 129 + 128]
                    # col 0:128 = c  (gpsimd)
                    nc.gpsimd.tensor_copy(st[:, 0:128], cchunk)
                    # col 128:256 = A = Araw * rcprow  (ACT copy-scale)
                    nc.scalar.activation(
                        st[:, 128:256], ab[:, 0:128], AF.Copy,
                        scale=rcprow[:, k:k + 1],
                    )
                    # col 256:384 = c*A  (DVE)
                    nc.vector.scalar_tensor_tensor(
                        st[:, 256:384], ab[:, 0:128], rcprow[:, k:k + 1],
                        cchunk, ALU.mult, ALU.mult,
                    )
                    # col 384:512 = c*Bm  (DVE)
                    nc.vector.scalar_tensor_tensor(
                        st[:, 384:512], ab[:, 128:256], rcprow[:, k:k + 1],
                        cchunk, ALU.mult, ALU.mult,
                    )
                    nc.sync.dma_start(out_ext[b, k * 128:(k + 1) * 128, :], st[:])

            if loop_n:
                with tc.For_i(0, loop_n, 1):
                    for b in range(BPC):
                        _batch(b)
            else:
                for b in range(BPC):
                    _batch(b)
    return nc


def _prep(c, q, c_mask, q_mask, c_weight, q_weight, cq_weight, bias):
    c = np.ascontiguousarray(np.asarray(c, dtype=np.float32))
    q = np.ascontiguousarray(np.asarray(q, dtype=np.float32))
    c_mask = np.asarray(c_mask)
    q_mask = np.asarray(q_mask)
    q_weight = np.asarray(q_weight, dtype=np.float32)
    cq_weight = np.asarray(cq_weight, dtype=np.float32)

    # host-side prep (tiny). NOTE: c@c_weight and bias cancel in softmax_j.
    s1 = (q.reshape(-1, D) @ q_weight).reshape(B, QL)          # (B, 128)
    R = s1 + np.where(q_mask > 0, 0.0, MASK_NEG).astype(np.float32)
    cmb = np.where(c_mask > 0, 0.0, MASK_NEG).astype(np.float32)  # (B, 1024)
    cT = np.ascontiguousarray(c.transpose(0, 2, 1)).astype(ml_dtypes.bfloat16)
    qmodT = np.ascontiguousarray(
        (q * cq_weight.reshape(1, 1, D)).transpose(0, 2, 1)
    ).astype(ml_dtypes.bfloat16)
    qT_rows = q.astype(ml_dtypes.bfloat16)                     # (B, 128, 128) [j, e]
    qq = np.concatenate([qmodT, qT_rows], axis=2)              # (B, 128, 256)

    in_maps = []
    for core in range(N_CORES):
        sl = slice(core * BPC, (core + 1) * BPC)
        cmT = cmb[sl].reshape(BPC, NK, 128).transpose(2, 0, 1).reshape(128, BPC * NK)
        cmR = np.ascontiguousarray(
            np.concatenate([cmT, R[sl].T], axis=1)             # (128, 64+8)
        )
        in_maps.append({
            "c": c[sl],
            "cT": cT[sl],
            "qq": np.ascontiguousarray(qq[sl]),
            "cmR": cmR,
            "ident": np.eye(128, dtype=ml_dtypes.bfloat16),
        })
    return in_maps


def make_in_maps():
    """For the local test/compare harness only (imports reference)."""
    import reference
    inputs = {k: np.asarray(v) for k, v in reference.setup_inputs().items()}
    return _prep(**inputs)


def kernel(c, q, c_mask, q_mask, c_weight, q_weight, cq_weight, bias):
    global LAST_RESULTS
    in_maps = _prep(c, q, c_mask, q_mask, c_weight, q_weight, cq_weight, bias)
    os.environ["BASS_NEVER_TRACE"] = "1"  # no NTFF hook in this container
    nc = _build_graph()
    nc.finalize()
    res = run_bass_kernel_spmd(nc, in_maps, core_ids=list(range(N_CORES)))
    LAST_RESULTS = (nc, in_maps)
    return np.concatenate([res.results[i]["out"] for i in range(N_CORES)], axis=0)



# revision 2
# speedup vs baseline: 1.1305x; 1.1305x over previous
"""C2Q (BiDAF-style) attention kernel for 8 TRN2 NeuronCores — v2.

Pure data parallel: 64 batches, 8 per core. Per batch (reference):
    S = c @ c_w + (q @ q_w)^T + (c * cq_w) @ q^T + bias      (1024, 128)
    S1 = masked_softmax(S, q_mask, axis=j)
    S2 = masked_softmax(S1, c_mask, axis=i)
    A = S1 @ q ; Bm = S1 @ (S2^T @ c normalized)
    out = [c | A | c*A | c*Bm]                                (1024, 512)

softmax over j is invariant to per-i constants, so c @ c_w and bias cancel
in S1. Only R[j] = q @ q_w + log-mask(q_mask) survives.

v2 structure (vs baseline): all inputs host-packed into per-core contiguous
HBM tensors loaded with ONE big DMA each; c is bf16 everywhere; rowsum via
gpsimd partition_all_reduce + wide reciprocal + wide multiply produces S1T
directly, so downstream matmuls emit final A/B with no per-chunk scale ops;
staging uses wide strided ops; stores are 1MB contiguous per half-batch.

Device per batch:
    S^T half h: psum = qmodT.T @ ct[b,h]      (bf16 matmul, N=512)
    e0t = exp(S^T + R[j])                      ACT bias, psum->sbuf bf16
    rsum = partition_all_reduce(e0t)           Pool, [128,1024] all rows
    rcp  = 1/rsum ; s1t = e0t * rcp            DVE wide, bf16  (= S1^T)
    per half: 4 PE transposes -> s1 chunks in one [128,512] bf16 psum bank
              4 ACT exps (bias cmb chunk) -> g_all sbuf bf16
    traw[j,0:129] = sum_k g_k^T @ [c_k | 1]    8 bf16 matmuls, accum psum
    Ts = traw * (1/colsum) -> qq Ts slot       DVE
    per half: ab[128,1024] = s1t_k.T @ [q|Ts]  4 matmuls N=256 -> f32 psum
              st = [c | A | c*A | c*B] wide strided: Pool copy, ACT copy,
              DVE tensor_tensor x2 ; 1MB contiguous store per half
"""

import os
import numpy as np
import ml_dtypes

import concourse.bass as bass
import concourse.tile as tile
from concourse import bacc, mybir, bass_isa
from concourse.bass_utils import run_bass_kernel_spmd

F32 = mybir.dt.float32
BF16 = mybir.dt.bfloat16
AF = mybir.ActivationFunctionType
ALU = mybir.AluOpType

N_CORES = 8
B, CL, QL, D = 64, 1024, 128, 128
BPC = B // N_CORES          # batches per core
NK = CL // 128              # 128-row chunks per batch
MASK_NEG = -50.0            # exp(-50+eps) vanishes in f32 sums; in ACT range

LAST_RESULTS = None         # set by kernel() for test.py profiling


def _build_graph(loop_n=0):
    """loop_n=0: straight-line graph (production). loop_n=N>0: wrap the whole
    computation in a hardware For_i loop repeating it N times (timing only)."""
    nc = bacc.Bacc()

    cn_ext = nc.declare_dram_parameter("cn", [128, BPC * NK * 129], BF16, isOutput=False)
    ct_ext = nc.declare_dram_parameter("ct", [128, BPC * CL], BF16, isOutput=False)
    qq_ext = nc.declare_dram_parameter("qq", [128, BPC * 2 * QL], BF16, isOutput=False)
    cmR_ext = nc.declare_dram_parameter("cmR", [128, BPC * NK + BPC], F32, isOutput=False)
    id_ext = nc.declare_dram_parameter("ident", [128, 128], BF16, isOutput=False)
    out_ext = nc.declare_dram_parameter("out", [BPC, CL, 4 * D], F32, isOutput=True)

    with tile.TileContext(nc) as tc, nc.allow_low_precision(reason="bf16 softmax pipeline; validated vs reference"):
        with (
            tc.tile_pool(name="const", bufs=1) as const,
            tc.tile_pool(name="e0tp", bufs=2) as e0tp,
            tc.tile_pool(name="rsbp", bufs=2) as rsbp,
            tc.tile_pool(name="rcpp", bufs=2) as rcpp,
            tc.tile_pool(name="s1tp", bufs=2) as s1tp,
            tc.tile_pool(name="gp", bufs=2) as gp,
            tc.tile_pool(name="rsp", bufs=2) as rsp,
            tc.tile_pool(name="stg", bufs=3) as stg,
            tc.tile_pool(name="stp", bufs=1, space=bass.MemorySpace.PSUM) as stp,
            tc.tile_pool(name="tpq", bufs=2, space=bass.MemorySpace.PSUM) as tpqp,
            tc.tile_pool(name="trawp", bufs=1, space=bass.MemorySpace.PSUM) as trawp,
            tc.tile_pool(name="abp", bufs=2, space=bass.MemorySpace.PSUM) as abp,
        ):
            ident = const.tile([128, 128], BF16, tag="ident")
            nc.sync.dma_start(ident[:], id_ext[:])
            cmR = const.tile([128, BPC * NK + BPC], F32, tag="cmR")
            nc.sync.dma_start(cmR[:], cmR_ext[:])
            cn_t = const.tile([128, BPC * NK * 129], BF16, tag="cn")
            ct_t = const.tile([128, BPC * CL], BF16, tag="ct")
            qq_t = const.tile([128, BPC * 3 * QL], BF16, tag="qq")

            def _load_batch(b):
                # Per-batch slices of the resident tiles so batch 0's compute
                # starts after ~600KB instead of after all 4.7MB of loads.
                nc.sync.dma_start(
                    cn_t[:, b * NK * 129:(b + 1) * NK * 129],
                    cn_ext[:, b * NK * 129:(b + 1) * NK * 129],
                )
                nc.sync.dma_start(
                    ct_t[:, b * CL:(b + 1) * CL], ct_ext[:, b * CL:(b + 1) * CL]
                )
                nc.sync.dma_start(
                    qq_t[:, b * 3 * QL: b * 3 * QL + 2 * QL],
                    qq_ext[:, b * 2 * QL:(b + 1) * 2 * QL],
                )

            def _batch(b):
                qmod = qq_t[:, b * 3 * QL: b * 3 * QL + QL]
                qts = qq_t[:, b * 3 * QL + QL: (b + 1) * 3 * QL]
                ts_slot = qq_t[:, b * 3 * QL + 2 * QL: (b + 1) * 3 * QL]

                # S^T = qmodT.T @ cT ; e0t = exp(S^T + R[j])  [j, 1024] bf16
                e0t = e0tp.tile([128, CL], BF16, tag="e0t")
                for h in range(2):
                    sp = stp.tile([128, 512], F32, tag="sp")
                    nc.tensor.matmul(
                        sp[:], qmod, ct_t[:, b * CL + h * 512: b * CL + (h + 1) * 512]
                    )
                    nc.scalar.activation(
                        e0t[:, h * 512:(h + 1) * 512], sp[:], AF.Exp,
                        bias=cmR[:, BPC * NK + b: BPC * NK + b + 1],
                    )

                # rowsum over j (partitions), broadcast to all: [128, 1024]
                rsum = rsbp.tile([128, CL], BF16, tag="rsum")
                nc.gpsimd.partition_all_reduce(
                    rsum[:], e0t[:], 128, bass_isa.ReduceOp.add
                )
                rcp = rcpp.tile([128, CL], BF16, tag="rcp")
                nc.vector.reciprocal(rcp[:], rsum[:])
                s1t = s1tp.tile([128, CL], BF16, tag="s1t")
                nc.vector.tensor_tensor(s1t[:], e0t[:], rcp[:], ALU.mult)

                # transpose s1t chunks -> s1 natural; G = exp(s1 + cmb)
                g_all = gp.tile([128, CL], BF16, tag="g")
                for h in range(2):
                    tpq = tpqp.tile([128, 512], BF16, tag="tpq")
                    for kk in range(4):
                        k = h * 4 + kk
                        nc.tensor.transpose(
                            tpq[:, kk * 128:(kk + 1) * 128],
                            s1t[:, k * 128:(k + 1) * 128], ident[:],
                        )
                    for kk in range(4):
                        k = h * 4 + kk
                        nc.scalar.activation(
                            g_all[:, k * 128:(k + 1) * 128],
                            tpq[:, kk * 128:(kk + 1) * 128], AF.Exp,
                            bias=cmR[:, b * NK + k: b * NK + k + 1],
                        )

                # Traw[j, 0:129] = sum_k G_k^T @ [c_k | 1]  (col 128 = colsum)
                traw = trawp.tile([128, 129], F32, tag="traw")
                for k in range(NK):
                    nc.tensor.matmul(
                        traw[:, 0:129], g_all[:, k * 128:(k + 1) * 128],
                        cn_t[:, (b * NK + k) * 129: (b * NK + k + 1) * 129],
                        start=(k == 0), stop=(k == NK - 1),
                    )
                rcp2 = rsp.tile([128, 1], F32, tag="rcp2")
                nc.vector.reciprocal(rcp2[:], traw[:, 128:129])
                nc.vector.tensor_scalar_mul(ts_slot, traw[:, 0:128], rcp2[:])

                # [A|B] = s1t_k.T @ [q | Ts] ; stage and store per half
                for h in range(2):
                    ab = abp.tile([128, 1024], F32, tag="ab")
                    for kk in range(4):
                        k = h * 4 + kk
                        nc.tensor.matmul(
                            ab[:, kk * 256:(kk + 1) * 256],
                            s1t[:, k * 128:(k + 1) * 128], qts,
                        )
                    st = stg.tile([128, 2048], F32, tag="st")
                    stv = st[:].rearrange("p (k q) -> p k q", q=512)
                    abv = ab[:].rearrange("p (k d) -> p k d", d=256)
                    cnv = cn_t[:].rearrange("p (m e) -> p m e", e=129)[
                        :, b * NK + h * 4: b * NK + h * 4 + 4, :
                    ]
                    # col 0:128 = c  (gpsimd, bf16 -> f32)
                    nc.gpsimd.tensor_copy(stv[:, :, 0:128], cnv[:, :, 0:128])
                    # col 128:256 = A  (ACT copy from psum)
                    nc.scalar.activation(stv[:, :, 128:256], abv[:, :, 0:128], AF.Copy)
                    # col 256:384 = c*A  (DVE)
                    nc.vector.tensor_tensor(
                        stv[:, :, 256:384], abv[:, :, 0:128], cnv[:, :, 0:128], ALU.mult
                    )
                    # col 384:512 = c*Bm  (DVE)
                    nc.vector.tensor_tensor(
                        stv[:, :, 384:512], abv[:, :, 128:256], cnv[:, :, 0:128], ALU.mult
                    )
                    nc.sync.dma_start(
                        out_ext[b, h * 512:(h + 1) * 512, :].rearrange(
                            "(k p) q -> p k q", p=128
                        ),
                        stv,
                    )

            if loop_n:
                with tc.For_i(0, loop_n, 1):
                    for b in range(BPC):
                        _load_batch(b)
                        _batch(b)
            else:
                for b in range(BPC):
                    _load_batch(b)
                    _batch(b)
    return nc


def _prep(c, q, c_mask, q_mask, c_weight, q_weight, cq_weight, bias):
    c = np.ascontiguousarray(np.asarray(c, dtype=np.float32))
    q = np.ascontiguousarray(np.asarray(q, dtype=np.float32))
    c_mask = np.asarray(c_mask)
    q_mask = np.asarray(q_mask)
    q_weight = np.asarray(q_weight, dtype=np.float32)
    cq_weight = np.asarray(cq_weight, dtype=np.float32)

    # host-side prep (tiny). NOTE: c@c_weight and bias cancel in softmax_j.
    s1 = (q.reshape(-1, D) @ q_weight).reshape(B, QL)          # (B, 128)
    R = s1 + np.where(q_mask > 0, 0.0, MASK_NEG).astype(np.float32)
    cmb = np.where(c_mask > 0, 0.0, MASK_NEG).astype(np.float32)  # (B, 1024)

    c_bf = c.astype(ml_dtypes.bfloat16)
    # cn: [128, BPC*NK*129] per core; block (b,k): [c rows k*128+p | 1.0]
    cn_all = np.ones((B, NK, 128, 129), dtype=ml_dtypes.bfloat16)
    cn_all[:, :, :, 0:128] = c_bf.reshape(B, NK, 128, D)
    # ct: [128(d), B*CL]
    ct_all = c_bf.transpose(2, 0, 1)                            # (128, B, CL)
    # qq: per batch [qmodT(128) | qT(128)]
    qmodT = np.ascontiguousarray(
        (q * cq_weight.reshape(1, 1, D)).transpose(0, 2, 1)
    ).astype(ml_dtypes.bfloat16)                                # (B, 128, 128)
    qT_rows = q.astype(ml_dtypes.bfloat16)                      # (B, 128, 128)
    qq_all = np.concatenate([qmodT, qT_rows], axis=2)           # (B, 128, 256)

    in_maps = []
    for core in range(N_CORES):
        sl = slice(core * BPC, (core + 1) * BPC)
        cn = np.ascontiguousarray(
            cn_all[sl].transpose(2, 0, 1, 3).reshape(128, BPC * NK * 129)
        )
        ct = np.ascontiguousarray(ct_all[:, sl].reshape(128, BPC * CL))
        qq = np.ascontiguousarray(
            qq_all[sl].transpose(1, 0, 2).reshape(128, BPC * 2 * QL)
        )
        cmT = cmb[sl].reshape(BPC, NK, 128).transpose(2, 0, 1).reshape(128, BPC * NK)
        cmR = np.ascontiguousarray(
            np.concatenate([cmT, R[sl].T], axis=1)              # (128, 64+8)
        ).astype(np.float32)
        in_maps.append({
            "cn": cn,
            "ct": ct,
            "qq": qq,
            "cmR": cmR,
            "ident": np.eye(128, dtype=ml_dtypes.bfloat16),
        })
    return in_maps


def make_in_maps():
    """For the local test/compare harness only (imports reference)."""
    import reference
    inputs = {k: np.asarray(v) for k, v in reference.setup_inputs().items()}
    return _prep(**inputs)


def kernel(c, q, c_mask, q_mask, c_weight, q_weight, cq_weight, bias):
    global LAST_RESULTS
    in_maps = _prep(c, q, c_mask, q_mask, c_weight, q_weight, cq_weight, bias)
    os.environ["BASS_NEVER_TRACE"] = "1"  # no NTFF hook in this container
    nc = _build_graph()
    nc.finalize()
    res = run_bass_kernel_spmd(nc, in_maps, core_ids=list(range(N_CORES)))
    LAST_RESULTS = (nc, in_maps)
    return np.concatenate([res.results[i]["out"] for i in range(N_CORES)], axis=0)


# revision 3
# speedup vs baseline: 1.2859x; 1.1374x over previous
"""C2Q (BiDAF-style) attention kernel for 8 TRN2 NeuronCores — v2.

Pure data parallel: 64 batches, 8 per core. Per batch (reference):
    S = c @ c_w + (q @ q_w)^T + (c * cq_w) @ q^T + bias      (1024, 128)
    S1 = masked_softmax(S, q_mask, axis=j)
    S2 = masked_softmax(S1, c_mask, axis=i)
    A = S1 @ q ; Bm = S1 @ (S2^T @ c normalized)
    out = [c | A | c*A | c*Bm]                                (1024, 512)

softmax over j is invariant to per-i constants, so c @ c_w and bias cancel
in S1. Only R[j] = q @ q_w + log-mask(q_mask) survives.

v2 structure (vs baseline): all inputs host-packed into per-core contiguous
HBM tensors loaded with ONE big DMA each; c is bf16 everywhere; rowsum via
gpsimd partition_all_reduce + wide reciprocal + wide multiply produces S1T
directly, so downstream matmuls emit final A/B with no per-chunk scale ops;
staging uses wide strided ops; stores are 1MB contiguous per half-batch.

Device per batch:
    S^T half h: psum = qmodT.T @ ct[b,h]      (bf16 matmul, N=512)
    e0t = exp(S^T + R[j])                      ACT bias, psum->sbuf bf16
    rsum = partition_all_reduce(e0t)           Pool, [128,1024] all rows
    rcp  = 1/rsum ; s1t = e0t * rcp            DVE wide, bf16  (= S1^T)
    per half: 4 PE transposes -> s1 chunks in one [128,512] bf16 psum bank
              4 ACT exps (bias cmb chunk) -> g_all sbuf bf16
    traw[j,0:129] = sum_k g_k^T @ [c_k | 1]    8 bf16 matmuls, accum psum
    Ts = traw * (1/colsum) -> qq Ts slot       DVE
    per half: ab[128,1024] = s1t_k.T @ [q|Ts]  4 matmuls N=256 -> f32 psum
              st = [c | A | c*A | c*B] wide strided: Pool copy, ACT copy,
              DVE tensor_tensor x2 ; 1MB contiguous store per half
"""

import os
import numpy as np
import ml_dtypes

import concourse.bass as bass
import concourse.tile as tile
from concourse import bacc, mybir, bass_isa
from concourse.bass_utils import run_bass_kernel_spmd

F32 = mybir.dt.float32
BF16 = mybir.dt.bfloat16
AF = mybir.ActivationFunctionType
ALU = mybir.AluOpType

N_CORES = 8
B, CL, QL, D = 64, 1024, 128, 128
BPC = B // N_CORES          # batches per core
NK = CL // 128              # 128-row chunks per batch
MASK_NEG = -50.0            # exp(-50+eps) vanishes in f32 sums; in ACT range

LAST_RESULTS = None         # set by kernel() for test.py profiling


def _build_graph(loop_n=0):
    """loop_n=0: straight-line graph (production). loop_n=N>0: wrap the whole
    computation in a hardware For_i loop repeating it N times (timing only)."""
    nc = bacc.Bacc()

    cn_ext = nc.declare_dram_parameter("cn", [128, BPC * NK * 129], BF16, isOutput=False)
    ct_ext = nc.declare_dram_parameter("ct", [128, BPC * CL], BF16, isOutput=False)
    qq_ext = nc.declare_dram_parameter("qq", [128, BPC * 2 * QL], BF16, isOutput=False)
    cmR_ext = nc.declare_dram_parameter("cmR", [128, BPC * NK + BPC], F32, isOutput=False)
    id_ext = nc.declare_dram_parameter("ident", [128, 128], BF16, isOutput=False)
    idf_ext = nc.declare_dram_parameter("identf", [128, 128], F32, isOutput=False)
    us_ext = nc.declare_dram_parameter("usel", [8, NK * 128], BF16, isOutput=False)
    out_ext = nc.declare_dram_parameter("out", [BPC, CL, 4 * D], F32, isOutput=True)

    with tile.TileContext(nc) as tc, nc.allow_low_precision(reason="bf16 softmax pipeline; validated vs reference"):
        with (
            tc.tile_pool(name="const", bufs=1) as const,
            tc.tile_pool(name="cnp", bufs=3) as cnp,
            tc.tile_pool(name="ctp", bufs=3) as ctp,
            tc.tile_pool(name="qqp", bufs=3) as qqp,
            tc.tile_pool(name="e0tp", bufs=2) as e0tp,
            tc.tile_pool(name="s1tp", bufs=2) as s1tp,
            tc.tile_pool(name="rsbsb", bufs=2) as rsbsb,
            tc.tile_pool(name="gp", bufs=2) as gp,
            tc.tile_pool(name="rsp", bufs=2) as rsp,
            tc.tile_pool(name="stg", bufs=3) as stg,
            tc.tile_pool(name="stp", bufs=1, space=bass.MemorySpace.PSUM) as stp,
            tc.tile_pool(name="tpq", bufs=2, space=bass.MemorySpace.PSUM) as tpqp,
            tc.tile_pool(name="trawp", bufs=1, space=bass.MemorySpace.PSUM) as trawp,
            tc.tile_pool(name="abp", bufs=3, space=bass.MemorySpace.PSUM) as abp,
            tc.tile_pool(name="bcp", bufs=1, space=bass.MemorySpace.PSUM) as bcp,
        ):
            ident = const.tile([128, 128], BF16, tag="ident")
            nc.sync.dma_start(ident[:], id_ext[:])
            identf = const.tile([128, 128], F32, tag="identf")
            nc.sync.dma_start(identf[:], idf_ext[:])
            usel = const.tile([8, NK * 128], BF16, tag="usel")
            nc.sync.dma_start(usel[:], us_ext[:])
            cmR = const.tile([128, BPC * NK + BPC], F32, tag="cmR")
            nc.sync.dma_start(cmR[:], cmR_ext[:])
            def _load_batch(b):
                # Per-batch double-buffered tiles: loads for iteration i+1 /
                # batch b+1 overlap compute without write-after-read stalls.
                ct_t = ctp.tile([128, CL], BF16, tag="ct")
                nc.sync.dma_start(ct_t[:], ct_ext[:, b * CL:(b + 1) * CL])
                qq_t = qqp.tile([128, 3 * QL], BF16, tag="qq")
                nc.sync.dma_start(
                    qq_t[:, 0:2 * QL], qq_ext[:, b * 2 * QL:(b + 1) * 2 * QL]
                )
                cn_t = cnp.tile([128, NK * 129], BF16, tag="cn")
                nc.sync.dma_start(cn_t[:], cn_ext[:, b * NK * 129:(b + 1) * NK * 129])
                return cn_t, ct_t, qq_t

            def _batch(b, cn_t, ct_t, qq_t):
                qmod = qq_t[:, 0:QL]
                qts = qq_t[:, QL:3 * QL]
                ts_slot = qq_t[:, 2 * QL:3 * QL]

                # S^T = qmodT.T @ cT ; e0t = exp(S^T + R[j])  [j, 1024] bf16
                e0t = e0tp.tile([128, CL], BF16, tag="e0t")
                for h in range(2):
                    sp = stp.tile([128, 512], F32, tag="sp")
                    nc.tensor.matmul(
                        sp[:], qmod, ct_t[:, h * 512:(h + 1) * 512]
                    )
                    nc.scalar.activation(
                        e0t[:, h * 512:(h + 1) * 512], sp[:], AF.Exp,
                        bias=cmR[:, BPC * NK + b: BPC * NK + b + 1],
                    )

                # transpose RAW e0t chunks -> E0 natural (psum); rowsum via
                # cheap chunked DVE reduces; G = exp(E0*rcprow + cmb)
                rowsum = rsp.tile([128, NK], F32, tag="rowsum")
                rcpf = rsp.tile([128, NK], F32, tag="rcpf")
                g_all = gp.tile([128, CL], BF16, tag="g")
                tpqs = []
                for h in range(2):
                    tpq = tpqp.tile([128, 512], BF16, tag="tpq")
                    tpqs.append(tpq)
                    for kk in range(4):
                        k = h * 4 + kk
                        nc.tensor.transpose(
                            tpq[:, kk * 128:(kk + 1) * 128],
                            e0t[:, k * 128:(k + 1) * 128], ident[:],
                        )
                    for kk in range(4):
                        k = h * 4 + kk
                        nc.vector.tensor_reduce(
                            rowsum[:, k:k + 1], tpq[:, kk * 128:(kk + 1) * 128],
                            mybir.AxisListType.X, ALU.add,
                        )
                nc.vector.reciprocal(rcpf[:], rowsum[:])
                for h in range(2):
                    for kk in range(4):
                        k = h * 4 + kk
                        nc.scalar.activation(
                            g_all[:, k * 128:(k + 1) * 128],
                            tpqs[h][:, kk * 128:(kk + 1) * 128], AF.Exp,
                            bias=cmR[:, b * NK + k: b * NK + k + 1],
                            scale=rcpf[:, k:k + 1],
                        )

                # broadcast rcprow into the transposed domain via PE:
                # transpose [128,8] -> [8,128] (psum corner), copy to sbuf,
                # then selector matmuls replicate row k across all partitions.
                s1t = s1tp.tile([128, CL], BF16, tag="s1t")
                rsb = rsbsb.tile([8, 128], BF16, tag="rsb")
                for h in range(2):
                    bc = bcp.tile([128, 512], F32, tag="bc")
                    if h == 0:
                        nc.tensor.transpose(bc[0:8, 384:512], rcpf[:], identf[:])
                        nc.vector.tensor_scalar_add(rsb[:], bc[0:8, 384:512], 0.0)
                    for kk in range(4):
                        k = h * 4 + kk
                        nc.tensor.matmul(
                            bc[:, kk * 128:(kk + 1) * 128],
                            usel[:, k * 128:(k + 1) * 128], rsb[:],
                        )
                    nc.vector.tensor_tensor(
                        s1t[:, h * 512:(h + 1) * 512], e0t[:, h * 512:(h + 1) * 512],
                        bc[:], ALU.mult,
                    )

                # Traw[j, 0:129] = sum_k G_k^T @ [c_k | 1]  (col 128 = colsum)
                traw = trawp.tile([128, 129], F32, tag="traw")
                for k in range(NK):
                    nc.tensor.matmul(
                        traw[:, 0:129], g_all[:, k * 128:(k + 1) * 128],
                        cn_t[:, k * 129:(k + 1) * 129],
                        start=(k == 0), stop=(k == NK - 1),
                    )
                rcp2 = rsp.tile([128, 1], F32, tag="rcp2")
                nc.vector.reciprocal(rcp2[:], traw[:, 128:129])
                nc.vector.tensor_scalar_mul(ts_slot, traw[:, 0:128], rcp2[:])

                # [A|B] = s1t_k.T @ [q | Ts] ; stage and store per half
                for h in range(2):
                    st = stg.tile([128, 2048], F32, tag="st")
                    stv = st[:].rearrange("p (k q) -> p k q", q=512)
                    cnv = cn_t[:].rearrange("p (m e) -> p m e", e=129)[
                        :, h * 4: h * 4 + 4, :
                    ]
                    # col 0:128 = c  (gpsimd, bf16 -> f32)
                    nc.gpsimd.tensor_copy(stv[:, :, 0:128], cnv[:, :, 0:128])
                    for g2 in range(2):
                        ab = abp.tile([128, 512], F32, tag="ab")
                        for kk in range(2):
                            k = h * 4 + g2 * 2 + kk
                            nc.tensor.matmul(
                                ab[:, kk * 256:(kk + 1) * 256],
                                s1t[:, k * 128:(k + 1) * 128], qts,
                            )
                        sv = stv[:, g2 * 2:(g2 + 1) * 2, :]
                        av = ab[:].rearrange("p (k d) -> p k d", d=256)
                        cv = cnv[:, g2 * 2:(g2 + 1) * 2, 0:128]
                        # col 128:256 = A  (ACT copy from psum)
                        nc.scalar.activation(sv[:, :, 128:256], av[:, :, 0:128], AF.Copy)
                        # col 256:384 = c*A ; col 384:512 = c*Bm  (DVE)
                        nc.vector.tensor_tensor(
                            sv[:, :, 256:384], av[:, :, 0:128], cv, ALU.mult
                        )
                        nc.vector.tensor_tensor(
                            sv[:, :, 384:512], av[:, :, 128:256], cv, ALU.mult
                        )
                    nc.sync.dma_start(
                        out_ext[b, h * 512:(h + 1) * 512, :].rearrange(
                            "(k p) q -> p k q", p=128
                        ),
                        stv,
                    )

            if loop_n:
                with tc.For_i(0, loop_n, 1):
                    for b in range(BPC):
                        _batch(b, *_load_batch(b))
            else:
                for b in range(BPC):
                    _batch(b, *_load_batch(b))
    return nc


def _prep(c, q, c_mask, q_mask, c_weight, q_weight, cq_weight, bias):
    c = np.ascontiguousarray(np.asarray(c, dtype=np.float32))
    q = np.ascontiguousarray(np.asarray(q, dtype=np.float32))
    c_mask = np.asarray(c_mask)
    q_mask = np.asarray(q_mask)
    q_weight = np.asarray(q_weight, dtype=np.float32)
    cq_weight = np.asarray(cq_weight, dtype=np.float32)

    # host-side prep (tiny). NOTE: c@c_weight and bias cancel in softmax_j.
    s1 = (q.reshape(-1, D) @ q_weight).reshape(B, QL)          # (B, 128)
    R = s1 + np.where(q_mask > 0, 0.0, MASK_NEG).astype(np.float32)
    cmb = np.where(c_mask > 0, 0.0, MASK_NEG).astype(np.float32)  # (B, 1024)

    c_bf = c.astype(ml_dtypes.bfloat16)
    # cn: [128, BPC*NK*129] per core; block (b,k): [c rows k*128+p | 1.0]
    cn_all = np.ones((B, NK, 128, 129), dtype=ml_dtypes.bfloat16)
    cn_all[:, :, :, 0:128] = c_bf.reshape(B, NK, 128, D)
    # ct: [128(d), B*CL]
    ct_all = c_bf.transpose(2, 0, 1)                            # (128, B, CL)
    # qq: per batch [qmodT(128) | qT(128)]
    qmodT = np.ascontiguousarray(
        (q * cq_weight.reshape(1, 1, D)).transpose(0, 2, 1)
    ).astype(ml_dtypes.bfloat16)                                # (B, 128, 128)
    qT_rows = q.astype(ml_dtypes.bfloat16)                      # (B, 128, 128)
    qq_all = np.concatenate([qmodT, qT_rows], axis=2)           # (B, 128, 256)

    in_maps = []
    for core in range(N_CORES):
        sl = slice(core * BPC, (core + 1) * BPC)
        cn = np.ascontiguousarray(
            cn_all[sl].transpose(2, 0, 1, 3).reshape(128, BPC * NK * 129)
        )
        ct = np.ascontiguousarray(ct_all[:, sl].reshape(128, BPC * CL))
        qq = np.ascontiguousarray(
            qq_all[sl].transpose(1, 0, 2).reshape(128, BPC * 2 * QL)
        )
        cmT = cmb[sl].reshape(BPC, NK, 128).transpose(2, 0, 1).reshape(128, BPC * NK)
        cmR = np.ascontiguousarray(
            np.concatenate([cmT, R[sl].T], axis=1)              # (128, 64+8)
        ).astype(np.float32)
        usel = np.zeros((8, NK * 128), dtype=ml_dtypes.bfloat16)
        for k in range(NK):
            usel[k, k * 128:(k + 1) * 128] = 1.0
        in_maps.append({
            "cn": cn,
            "ct": ct,
            "qq": qq,
            "cmR": cmR,
            "ident": np.eye(128, dtype=ml_dtypes.bfloat16),
            "identf": np.eye(128, dtype=np.float32),
            "usel": usel,
        })
    return in_maps


def make_in_maps():
    """For the local test/compare harness only (imports reference)."""
    import reference
    inputs = {k: np.asarray(v) for k, v in reference.setup_inputs().items()}
    return _prep(**inputs)


def kernel(c, q, c_mask, q_mask, c_weight, q_weight, cq_weight, bias):
    global LAST_RESULTS
    in_maps = _prep(c, q, c_mask, q_mask, c_weight, q_weight, cq_weight, bias)
    os.environ["BASS_NEVER_TRACE"] = "1"  # no NTFF hook in this container
    nc = _build_graph()
    nc.finalize()
    res = run_bass_kernel_spmd(nc, in_maps, core_ids=list(range(N_CORES)))
    LAST_RESULTS = (nc, in_maps)
    return np.concatenate([res.results[i]["out"] for i in range(N_CORES)], axis=0)


# revision 4
# speedup vs baseline: 1.6668x; 1.2963x over previous
"""C2Q (BiDAF-style) attention kernel for 8 TRN2 NeuronCores — v2.

Pure data parallel: 64 batches, 8 per core. Per batch (reference):
    S = c @ c_w + (q @ q_w)^T + (c * cq_w) @ q^T + bias      (1024, 128)
    S1 = masked_softmax(S, q_mask, axis=j)
    S2 = masked_softmax(S1, c_mask, axis=i)
    A = S1 @ q ; Bm = S1 @ (S2^T @ c normalized)
    out = [c | A | c*A | c*Bm]                                (1024, 512)

softmax over j is invariant to per-i constants, so c @ c_w and bias cancel
in S1. Only R[j] = q @ q_w + log-mask(q_mask) survives.

v2 structure (vs baseline): all inputs host-packed into per-core contiguous
HBM tensors loaded with ONE big DMA each; c is bf16 everywhere; rowsum via
gpsimd partition_all_reduce + wide reciprocal + wide multiply produces S1T
directly, so downstream matmuls emit final A/B with no per-chunk scale ops;
staging uses wide strided ops; stores are 1MB contiguous per half-batch.

Device per batch:
    S^T half h: psum = qmodT.T @ ct[b,h]      (bf16 matmul, N=512)
    e0t = exp(S^T + R[j])                      ACT bias, psum->sbuf bf16
    rsum = partition_all_reduce(e0t)           Pool, [128,1024] all rows
    rcp  = 1/rsum ; s1t = e0t * rcp            DVE wide, bf16  (= S1^T)
    per half: 4 PE transposes -> s1 chunks in one [128,512] bf16 psum bank
              4 ACT exps (bias cmb chunk) -> g_all sbuf bf16
    traw[j,0:129] = sum_k g_k^T @ [c_k | 1]    8 bf16 matmuls, accum psum
    Ts = traw * (1/colsum) -> qq Ts slot       DVE
    per half: ab[128,1024] = s1t_k.T @ [q|Ts]  4 matmuls N=256 -> f32 psum
              st = [c | A | c*A | c*B] wide strided: Pool copy, ACT copy,
              DVE tensor_tensor x2 ; 1MB contiguous store per half
"""

import os
import numpy as np
import ml_dtypes

import concourse.bass as bass
import concourse.tile as tile
from concourse import bacc, mybir, bass_isa
from concourse.bass_utils import run_bass_kernel_spmd

F32 = mybir.dt.float32
BF16 = mybir.dt.bfloat16
AF = mybir.ActivationFunctionType
ALU = mybir.AluOpType

N_CORES = 8
B, CL, QL, D = 64, 1024, 128, 128
BPC = B // N_CORES          # batches per core
NK = CL // 128              # 128-row chunks per batch
MASK_NEG = -50.0            # exp(-50+eps) vanishes in f32 sums; in ACT range

LAST_RESULTS = None         # set by kernel() for test.py profiling


def _build_graph(loop_n=0):
    """loop_n=0: straight-line graph (production). loop_n=N>0: wrap the whole
    computation in a hardware For_i loop repeating it N times (timing only)."""
    nc = bacc.Bacc()

    # packed per-batch blocks: [ct(1024) | qmodT(128) | qT(128) | cn(1032)]
    PKB = CL + 2 * QL + NK * 129
    pk_ext = nc.declare_dram_parameter("pk", [128, BPC * PKB], BF16, isOutput=False)
    cmR_ext = nc.declare_dram_parameter("cmR", [128, BPC * NK + BPC], F32, isOutput=False)
    id_ext = nc.declare_dram_parameter("ident", [128, 128], BF16, isOutput=False)
    idf_ext = nc.declare_dram_parameter("identf", [128, 128], F32, isOutput=False)
    us_ext = nc.declare_dram_parameter("usel", [8, NK * 128], BF16, isOutput=False)
    out_ext = nc.declare_dram_parameter("out", [BPC, CL, 4 * D], F32, isOutput=True)

    with tile.TileContext(nc) as tc, nc.allow_low_precision(reason="bf16 softmax pipeline; validated vs reference"):
        with (
            tc.tile_pool(name="const", bufs=1) as const,
            tc.tile_pool(name="cnp", bufs=3) as cnp,
            tc.tile_pool(name="e0tp", bufs=3) as e0tp,
            tc.tile_pool(name="s1tp", bufs=3) as s1tp,
            tc.tile_pool(name="rsbsb", bufs=3) as rsbsb,
            tc.tile_pool(name="gp", bufs=3) as gp,
            tc.tile_pool(name="rsp", bufs=4) as rsp,
            tc.tile_pool(name="stg", bufs=4) as stg,
            tc.tile_pool(name="stp", bufs=1, space=bass.MemorySpace.PSUM) as stp,
            tc.tile_pool(name="tpq", bufs=2, space=bass.MemorySpace.PSUM) as tpqp,
            tc.tile_pool(name="trawp", bufs=1, space=bass.MemorySpace.PSUM) as trawp,
            tc.tile_pool(name="abp", bufs=3, space=bass.MemorySpace.PSUM) as abp,
            tc.tile_pool(name="bcp", bufs=1, space=bass.MemorySpace.PSUM) as bcp,
        ):
            ident = const.tile([128, 128], BF16, tag="ident")
            nc.sync.dma_start(ident[:], id_ext[:])
            identf = const.tile([128, 128], F32, tag="identf")
            nc.sync.dma_start(identf[:], idf_ext[:])
            usel = const.tile([8, NK * 128], BF16, tag="usel")
            nc.sync.dma_start(usel[:], us_ext[:])
            cmR = const.tile([128, BPC * NK + BPC], F32, tag="cmR")
            nc.sync.dma_start(cmR[:], cmR_ext[:])
            def _load_batch(b):
                # One packed tile per batch: [ct | qmod | qT | Ts-slot | cn].
                # Two DMAs: [ct|qmod|qT] (contiguous) and cn (contiguous),
                # leaving the device-written Ts slot between them.
                pk_t = cnp.tile([128, CL + 3 * QL + NK * 129], BF16, tag="pk")
                nc.sync.dma_start(
                    pk_t[:, 0:CL + 2 * QL],
                    pk_ext[:, b * PKB: b * PKB + CL + 2 * QL],
                )
                nc.sync.dma_start(
                    pk_t[:, CL + 3 * QL:],
                    pk_ext[:, b * PKB + CL + 2 * QL:(b + 1) * PKB],
                )
                ct_t = pk_t[:, 0:CL]
                qq_t = pk_t[:, CL:CL + 3 * QL]
                cn_t = pk_t[:, CL + 3 * QL:]
                return cn_t, ct_t, qq_t

            def _batch(b, cn_t, ct_t, qq_t):
                qmod = qq_t[:, 0:QL]
                qts = qq_t[:, QL:3 * QL]
                ts_slot = qq_t[:, 2 * QL:3 * QL]

                # S^T = qmodT.T @ cT ; e0t = exp(S^T + R[j])  [j, 1024] bf16
                e0t = e0tp.tile([128, CL], BF16, tag="e0t")
                for h in range(2):
                    sp = stp.tile([128, 512], F32, tag="sp")
                    nc.tensor.matmul(
                        sp[:], qmod, ct_t[:, h * 512:(h + 1) * 512]
                    )
                    nc.scalar.activation(
                        e0t[:, h * 512:(h + 1) * 512], sp[:], AF.Exp,
                        bias=cmR[:, BPC * NK + b: BPC * NK + b + 1],
                    )

                # transpose RAW e0t chunks -> E0 natural (psum); rowsum via
                # cheap chunked DVE reduces; G = exp(E0*rcprow + cmb)
                rowsum = rsp.tile([128, NK], F32, tag="rowsum")
                rcpf = rsp.tile([128, NK], F32, tag="rcpf")
                g_all = gp.tile([128, CL], BF16, tag="g")
                tpqs = []
                for h in range(2):
                    tpq = tpqp.tile([128, 512], BF16, tag="tpq")
                    tpqs.append(tpq)
                    for kk in range(4):
                        k = h * 4 + kk
                        nc.tensor.transpose(
                            tpq[:, kk * 128:(kk + 1) * 128],
                            e0t[:, k * 128:(k + 1) * 128], ident[:],
                        )
                    for kk in range(4):
                        k = h * 4 + kk
                        nc.vector.tensor_reduce(
                            rowsum[:, k:k + 1], tpq[:, kk * 128:(kk + 1) * 128],
                            mybir.AxisListType.X, ALU.add,
                        )
                nc.vector.reciprocal(rcpf[:], rowsum[:])

                # broadcast rcprow into the transposed domain via PE:
                # transpose [128,8] -> [8,128] (psum corner), copy to sbuf,
                # then selector matmuls replicate row k across all partitions.
                s1t = s1tp.tile([128, CL], BF16, tag="s1t")
                rsb = rsbsb.tile([8, 128], BF16, tag="rsb")
                for h in range(2):
                    bc = bcp.tile([128, 512], F32, tag="bc")
                    if h == 0:
                        nc.tensor.transpose(bc[0:8, 384:512], rcpf[:], identf[:])
                        nc.vector.tensor_scalar_add(rsb[:], bc[0:8, 384:512], 0.0)
                    for kk in range(4):
                        k = h * 4 + kk
                        nc.tensor.matmul(
                            bc[:, kk * 128:(kk + 1) * 128],
                            usel[:, k * 128:(k + 1) * 128], rsb[:],
                        )
                    nc.vector.tensor_tensor(
                        s1t[:, h * 512:(h + 1) * 512], e0t[:, h * 512:(h + 1) * 512],
                        bc[:], ALU.mult,
                    )

                for h in range(2):
                    for kk in range(4):
                        k = h * 4 + kk
                        nc.scalar.activation(
                            g_all[:, k * 128:(k + 1) * 128],
                            tpqs[h][:, kk * 128:(kk + 1) * 128], AF.Exp,
                            bias=cmR[:, b * NK + k: b * NK + k + 1],
                            scale=rcpf[:, k:k + 1],
                        )

                # Traw[j, 0:129] = sum_k G_k^T @ [c_k | 1]  (col 128 = colsum)
                traw = trawp.tile([128, 129], F32, tag="traw")
                for k in range(NK):
                    nc.tensor.matmul(
                        traw[:, 0:129], g_all[:, k * 128:(k + 1) * 128],
                        cn_t[:, k * 129:(k + 1) * 129],
                        start=(k == 0), stop=(k == NK - 1),
                    )
                rcp2 = rsp.tile([128, 1], F32, tag="rcp2")
                nc.vector.reciprocal(rcp2[:], traw[:, 128:129])
                nc.vector.tensor_scalar_mul(ts_slot, traw[:, 0:128], rcp2[:])

                # [A|B] = s1t_k.T @ [q | Ts] ; stage and store per half
                for h in range(2):
                    st = stg.tile([128, 2048], F32, tag="st")
                    stv = st[:].rearrange("p (k q) -> p k q", q=512)
                    cnv = cn_t.rearrange("p (m e) -> p m e", e=129)[
                        :, h * 4: h * 4 + 4, :
                    ]
                    # col 0:128 = c  (gpsimd, bf16 -> f32)
                    nc.gpsimd.tensor_copy(stv[:, :, 0:128], cnv[:, :, 0:128])
                    for g2 in range(2):
                        ab = abp.tile([128, 512], F32, tag="ab")
                        for kk in range(2):
                            k = h * 4 + g2 * 2 + kk
                            nc.tensor.matmul(
                                ab[:, kk * 256:(kk + 1) * 256],
                                s1t[:, k * 128:(k + 1) * 128], qts,
                            )
                        sv = stv[:, g2 * 2:(g2 + 1) * 2, :]
                        av = ab[:].rearrange("p (k d) -> p k d", d=256)
                        cv = cnv[:, g2 * 2:(g2 + 1) * 2, 0:128]
                        # col 128:256 = A (alternate ACT/DVE to balance load)
                        if g2 == 0:
                            nc.scalar.activation(sv[:, :, 128:256], av[:, :, 0:128], AF.Copy)
                        else:
                            nc.vector.tensor_scalar_add(sv[:, :, 128:256], av[:, :, 0:128], 0.0)
                        # col 256:384 = c*A ; col 384:512 = c*Bm  (DVE)
                        nc.vector.tensor_tensor(
                            sv[:, :, 256:384], av[:, :, 0:128], cv, ALU.mult
                        )
                        nc.vector.tensor_tensor(
                            sv[:, :, 384:512], av[:, :, 128:256], cv, ALU.mult
                        )
                    nc.sync.dma_start(
                        out_ext[b, h * 512:(h + 1) * 512, :].rearrange(
                            "(k p) q -> p k q", p=128
                        ),
                        stv,
                    )

            if loop_n:
                with tc.For_i(0, loop_n, 1):
                    for b in range(BPC):
                        _batch(b, *_load_batch(b))
            else:
                for b in range(BPC):
                    _batch(b, *_load_batch(b))
    return nc


def _prep(c, q, c_mask, q_mask, c_weight, q_weight, cq_weight, bias):
    c = np.ascontiguousarray(np.asarray(c, dtype=np.float32))
    q = np.ascontiguousarray(np.asarray(q, dtype=np.float32))
    c_mask = np.asarray(c_mask)
    q_mask = np.asarray(q_mask)
    q_weight = np.asarray(q_weight, dtype=np.float32)
    cq_weight = np.asarray(cq_weight, dtype=np.float32)

    # host-side prep (tiny). NOTE: c@c_weight and bias cancel in softmax_j.
    s1 = (q.reshape(-1, D) @ q_weight).reshape(B, QL)          # (B, 128)
    R = s1 + np.where(q_mask > 0, 0.0, MASK_NEG).astype(np.float32)
    cmb = np.where(c_mask > 0, 0.0, MASK_NEG).astype(np.float32)  # (B, 1024)

    c_bf = c.astype(ml_dtypes.bfloat16)
    # cn: [128, BPC*NK*129] per core; block (b,k): [c rows k*128+p | 1.0]
    cn_all = np.ones((B, NK, 128, 129), dtype=ml_dtypes.bfloat16)
    cn_all[:, :, :, 0:128] = c_bf.reshape(B, NK, 128, D)
    # ct: [128(d), B*CL]
    ct_all = c_bf.transpose(2, 0, 1)                            # (128, B, CL)
    # qq: per batch [qmodT(128) | qT(128)]
    qmodT = np.ascontiguousarray(
        (q * cq_weight.reshape(1, 1, D)).transpose(0, 2, 1)
    ).astype(ml_dtypes.bfloat16)                                # (B, 128, 128)
    qT_rows = q.astype(ml_dtypes.bfloat16)                      # (B, 128, 128)
    qq_all = np.concatenate([qmodT, qT_rows], axis=2)           # (B, 128, 256)

    in_maps = []
    for core in range(N_CORES):
        sl = slice(core * BPC, (core + 1) * BPC)
        cn = cn_all[sl].transpose(2, 0, 1, 3).reshape(128, BPC, NK * 129)
        ct = ct_all[:, sl].reshape(128, BPC, CL)
        qq = qq_all[sl].transpose(1, 0, 2).reshape(128, BPC, 2 * QL)
        pk = np.ascontiguousarray(
            np.concatenate([ct, qq, cn], axis=2).reshape(128, -1)
        )
        cmT = cmb[sl].reshape(BPC, NK, 128).transpose(2, 0, 1).reshape(128, BPC * NK)
        cmR = np.ascontiguousarray(
            np.concatenate([cmT, R[sl].T], axis=1)              # (128, 64+8)
        ).astype(np.float32)
        usel = np.zeros((8, NK * 128), dtype=ml_dtypes.bfloat16)
        for k in range(NK):
            usel[k, k * 128:(k + 1) * 128] = 1.0
        in_maps.append({
            "pk": pk,
            "cmR": cmR,
            "ident": np.eye(128, dtype=ml_dtypes.bfloat16),
            "identf": np.eye(128, dtype=np.float32),
            "usel": usel,
        })
    return in_maps


def make_in_maps():
    """For the local test/compare harness only (imports reference)."""
    import reference
    inputs = {k: np.asarray(v) for k, v in reference.setup_inputs().items()}
    return _prep(**inputs)


def kernel(c, q, c_mask, q_mask, c_weight, q_weight, cq_weight, bias):
    global LAST_RESULTS
    in_maps = _prep(c, q, c_mask, q_mask, c_weight, q_weight, cq_weight, bias)
    os.environ["BASS_NEVER_TRACE"] = "1"  # no NTFF hook in this container
    nc = _build_graph()
    nc.finalize()
    res = run_bass_kernel_spmd(nc, in_maps, core_ids=list(range(N_CORES)))
    LAST_RESULTS = (nc, in_maps)
    return np.concatenate([res.results[i]["out"] for i in range(N_CORES)], axis=0)
